# revision 30
# baseline (speedup 1.0000x reference)
"""Trainium2 Bass kernel for the (faithfully buggy) multi-head attention module.

Reference math (k = v = q due to the reference's reshape bug):
    q  = queries.reshape(B, S, H, D)
    qp = q @ Wq.T ; kp = q @ Wk.T ; vp = q @ Wv.T        (per-head, shared W)
    sim = qp @ kp.T / sqrt(D) ; attn = softmax(sim)
    out = (attn @ vp).reshape(B, S, E) @ Wo.T + bo

Folded form computed here (algebraically identical):
    A   = (1/sqrt(D)) * Wq.T @ Wk          ->  sim = q @ A @ q.T
    u   = attn @ q ;  av = u @ Wv.T        ->  attn @ vp == (attn @ q) @ Wv.T
    out = concat_h(av) @ Wo.T + bo

Sharding: 8 cores = (4 batches) x (2 halves of the 2048 query rows).
Each core computes its 1024 output rows for all 8 heads; keys/values span
the full 2048 rows of the core's batch. No collectives.

On-chip dataflow stays in the "transposed domain" (head_dim on
partitions) so no attention-matrix transposes are ever needed:
    qT[d, k]      : host-prepared transposed q (qtin, bf16)
    tT  = A-lhsT @ qT(own cols)                      [d', q]
    scT = qT(k-chunk)-lhsT @ tT                      [k, q]   (PSUM)
    eS  = exp(scT)  on ACT                           [k, q]   (SBUF)
    uT  = [q_chunk | ones]-lhsT @ eS                 [d'+1, q] (PSUM accum
          over k-chunks; row 64 = softmax denominator via the ones col)
    ut  = uT[:64, :] * bcast(recip(uT[64, :]))       (normalize, DVE+GPSIMD)
    avT = WvT-lhsT @ ut                              [d, q]  (head pairs
          packed into one PSUM tile at row offsets 0/64)
    out = aoT-pair-chunks-lhsT @ WoT-chunks (+ bo)   [s, e]

Matmuls run in bf16 (4x the fp32 PE rate) with fp32 PSUM accumulation.
With FP8UP=1 (default), the attn@q contraction (uT) runs in fp8-e4m3
DoubleRow mode — k-chunk PAIRS are contracted per instruction at 2x the
bf16 row rate (4x overall) — exp outputs are written as fp8 directly and
the q chunks arrive host-packed as [128, 2, H*HB] pair tiles.

Scores and attn@q are software-pipelined at k-chunk granularity so the
ACT engine (~1.1us per 128x1024 exp) and the PE stay concurrently busy;
each pair's Wv projection + softmax-normalize chain is deferred into the
next head's dense scores phase so it never bubbles the PE.
"""

import os

import numpy as np
import ml_dtypes

B, S, E = 4, 2048, 512
H, D = 8, 64
SH = S // 2          # rows per core
HB = D + 2           # per-head block: 64 q cols, 1 ones col, 1 pad (alignment)
NT_Q = SH // 128     # 8 own-row tiles
NT_K = S // 128      # 16 k chunks
NP_K = NT_K // 2     # 8 k-chunk pairs
NSP = SH // 512      # 2 q spans of 512
BF16 = ml_dtypes.bfloat16

FP8UP = bool(int(os.environ.get("KERNEL_FP8UP", "0")))

LAST_EXEC_NS = None
LAST_RESULTS = None


def _build_program():
    import concourse.bass as bass  # noqa: F401
    import concourse.mybir as mybir
    import concourse.tile as tile
    from concourse import bacc

    f32 = mybir.dt.float32
    bf = mybir.dt.bfloat16
    qdt = mybir.dt.float8e4 if FP8UP else bf
    DR = mybir.MatmulPerfMode.DoubleRow

    nc = bacc.Bacc("TRN2", target_bir_lowering=False, debug=False)

    # q chunk-pair tiles: row kp*128+p holds [chunk 2kp row p | chunk 2kp+1 row p]
    qpin = nc.dram_tensor("qpin", [SH, 2 * H * HB], qdt, kind="ExternalInput").ap()
    qtin = nc.dram_tensor("qtin", [E, S], bf, kind="ExternalInput").ap()
    a_dr = nc.dram_tensor("a_bf", [D, D], bf, kind="ExternalInput").ap()
    wvt_dr = nc.dram_tensor("wvt", [D, D], bf, kind="ExternalInput").ap()
    wot_dr = nc.dram_tensor("wot", [E, E], bf, kind="ExternalInput").ap()
    bob_dr = nc.dram_tensor("bob", [128, E], f32, kind="ExternalInput").ap()
    out_dr = nc.dram_tensor("out", [SH, E], f32, kind="ExternalOutput").ap()

    with tile.TileContext(nc) as tc:
        with (
            tc.tile_pool(name="singles", bufs=1) as singles,
            tc.tile_pool(name="work", bufs=3) as work,
            tc.tile_pool(name="es", bufs=10) as espool,
            tc.tile_pool(name="psS", bufs=2, space="PSUM") as psS,
            tc.tile_pool(name="psU", bufs=2, space="PSUM") as psU,
            tc.tile_pool(name="psB", bufs=2, space="PSUM") as psB,
        ):
            # critical-path inputs first: A, first heads' qT, q chunk pairs
            a_sb = singles.tile([D, D], bf, tag="a_sb")
            nc.sync.dma_start(out=a_sb, in_=a_dr)
            qT = []
            for h in range(H):
                qT.append(singles.tile([D, S], bf, tag=f"qT{h}", name=f"qT{h}"))
            for h in range(2):
                for r in range(0, D, 16):
                    nc.sync.dma_start(
                        out=qT[h][r : r + 16, :],
                        in_=qtin[h * D + r : h * D + r + 16, :],
                    )
            qs2 = []
            for kp in range(NP_K):
                t = singles.tile([128, 2, H * HB], qdt, tag=f"qs{kp}", name=f"qs{kp}")
                if kp < 2:
                    for r in range(0, 128, 64):
                        nc.sync.dma_start(
                            out=t[r : r + 64, :, :],
                            in_=qpin[kp * 128 + r : kp * 128 + r + 64, :],
                        )
                else:
                    nc.sync.dma_start(out=t, in_=qpin[kp * 128 : (kp + 1) * 128, :])
                qs2.append(t)
            for h in range(2, H):
                nc.sync.dma_start(out=qT[h], in_=qtin[h * D : (h + 1) * D, :])

            wvt_sb = singles.tile([D, D], bf, tag="wvt_sb")
            nc.sync.dma_start(out=wvt_sb, in_=wvt_dr)
            bob_sb = singles.tile([128, E], f32, tag="bob_sb")
            nc.sync.dma_start(out=bob_sb, in_=bob_dr)
            wot_sb = []
            for c in range(4):
                w = singles.tile([128, E], bf, tag=f"wot{c}", name=f"wot{c}")
                nc.sync.dma_start(out=w, in_=wot_dr[c * 128 : (c + 1) * 128, :])
                wot_sb.append(w)

            # attention outputs, head-PAIR packed: aoT[c][0:64] = head 2c,
            # aoT[c][64:128] = head 2c+1 (rows = e' = h*64+d).
            aoT = []
            for c in range(4):
                aoT.append(singles.tile([128, SH], bf, tag=f"aoT{c}", name=f"aoT{c}"))

            # per-head normalized uT spans, held until the pair's Wv matmul
            ut_tiles = {}

            def emit_uspan_epilogue(h, j, upj):
                # drain PSUM immediately, then normalize off the PE critical
                # path: ut = uT[:64] * bcast(1 / uT[64])
                uu = work.tile([65, 512], f32, tag="uu", bufs=4, name="uu")
                nc.vector.tensor_copy(uu, upj)
                d0 = work.tile([1, 512], f32, tag="d0", bufs=4, name="d0")
                nc.sync.dma_start(out=d0, in_=uu[64:65, :])
                rb = work.tile([D, 512], f32, tag="rb", bufs=4, name="rb")
                nc.gpsimd.partition_broadcast(rb, d0[0:1, :])
                rr = work.tile([D, 512], f32, tag="rr", bufs=4, name="rr")
                nc.vector.reciprocal_approx_fast(out=rr, in_=rb)
                ut = work.tile([D, 512], bf, tag="ut", bufs=6, name="ut")
                nc.vector.tensor_tensor(ut, uu[0:D, :], rr, mybir.AluOpType.mult)
                ut_tiles[(h, j)] = ut

            def emit_av_pair_span(hp, j):
                # Wv projection for both heads of the pair, packed in PSUM
                # rows 0:64 / 64:128, then one copy into the aoT pair tile.
                avp = psB.tile([128, 512], f32, tag="misc", name="avp")
                for hh in range(2):
                    nc.tensor.matmul(
                        avp[hh * D : (hh + 1) * D, :],
                        wvt_sb,
                        ut_tiles.pop((2 * hp + hh, j)),
                        start=True, stop=True,
                    )
                nc.vector.tensor_copy(aoT[hp][:, j * 512 : (j + 1) * 512], avp)

            def emit_tT_span(h, tts, j):
                # tT = (q_own @ A).T  [d', q]
                tp = psB.tile([D, 512], f32, tag="misc", name="tp")
                nc.tensor.matmul(
                    tp, a_sb, qT[h][:, j * 512 : (j + 1) * 512],
                    start=True, stop=True,
                )
                nc.vector.tensor_copy(tts[:, j * 512 : (j + 1) * 512], tp)

            # out-proj stage A: head pairs 0/1 contribution (+ bias), kept in
            # SBUF partials so only pairs 2/3 remain for the kernel tail.
            partials = {}

            def emit_outproj_b1_st(st):
                # accumulate head pair 2 onto the stage-A partial
                op = psB.tile([128, E], f32, tag="misc", name="opb")
                nc.tensor.matmul(
                    op, aoT[2][:, st * 128 : (st + 1) * 128], wot_sb[2],
                    start=True, stop=True,
                )
                nc.vector.tensor_add(partials[st], op, partials[st])

            def emit_outproj_a_st(st):
                op = psB.tile([128, E], f32, tag="misc", name="opa")
                for c in range(2):
                    nc.tensor.matmul(
                        op, aoT[c][:, st * 128 : (st + 1) * 128], wot_sb[c],
                        start=(c == 0), stop=(c == 1),
                    )
                pt = singles.tile([128, E], f32, tag=f"pt{st}", name=f"pt{st}")
                nc.vector.tensor_add(pt, op, bob_sb)
                partials[st] = pt

            # Software pipeline over heads: scores(h)/exp(h) interleaved with
            # attn@q of the same head lagging the pair's exp by one chunk;
            # each pair's Wv projection and out-proj stage A are deferred a
            # full head (ample slack for their DVE/DMA chains), and the next
            # head's tT is emitted mid-phase so head boundaries stay tight.
            pend_av = None
            pend_tail = None
            tts_all = [work.tile([D, SH], bf, tag=f"tts{h % 2}", name=f"tts{h}")
                       for h in range(H)]
            for j in range(NSP):
                emit_tT_span(0, tts_all[0], j)
            for h in range(H):
                tts = tts_all[h]
                es2 = []
                ups = [
                    psU.tile([D + 1, 512], f32, tag="up", name=f"up{j}")
                    for j in range(NSP)
                ]

                def emit_up(kp, s_sel=(0, 1), es2=es2, ups=ups, h=h):
                    if FP8UP:
                        for j in range(NSP):
                            nc.tensor.matmul(
                                ups[j],
                                qs2[kp][:, :, h * HB : h * HB + D + 1],
                                es2[kp][:, :, j * 512 : (j + 1) * 512],
                                start=(kp == 0), stop=(kp == NP_K - 1),
                                perf_mode=DR,
                            )
                    else:
                        for s in s_sel:
                            for j in range(NSP):
                                nc.tensor.matmul(
                                    ups[j],
                                    qs2[kp][:, s, h * HB : h * HB + D + 1],
                                    es2[kp][:, s, j * 512 : (j + 1) * 512],
                                    start=(kp == 0 and s == 0),
                                    stop=(kp == NP_K - 1 and s == 1),
                                )

                for kc in range(NT_K):
                    if kc == 1 and pend_tail is not None:
                        pend_tail()
                        pend_tail = None
                    if kc == 10 and pend_av is not None:
                        for j in range(NSP):
                            emit_av_pair_span(pend_av, j)
                        pend_av = None
                    if kc == 12 and h + 1 < H:
                        for j in range(NSP):
                            emit_tT_span(h + 1, tts_all[h + 1], j)
                    if h == 6 and kc in (8, 10, 12, 14):
                        st0 = (kc - 8)
                        emit_outproj_a_st(st0)
                        emit_outproj_a_st(st0 + 1)
                    if h == 7 and kc in (8, 10, 12, 14):
                        st0 = (kc - 8)
                        emit_outproj_b1_st(st0)
                        emit_outproj_b1_st(st0 + 1)
                    kp, s = divmod(kc, 2)
                    if s == 0:
                        es2.append(
                            espool.tile([128, 2, SH], qdt, tag="es", name=f"es{kp}")
                        )
                    sc = psS.tile([128, SH], f32, tag="sc")
                    for j in range(NSP):
                        nc.tensor.matmul(
                            sc[:, j * 512 : (j + 1) * 512],
                            qT[h][:, kc * 128 : (kc + 1) * 128],
                            tts[:, j * 512 : (j + 1) * 512],
                            start=True, stop=True,
                        )
                    nc.scalar.activation(
                        es2[kp][:, s, :], sc, mybir.ActivationFunctionType.Exp
                    )
                    # attn@q for an earlier pair, lagging its exp by one chunk
                    if kc >= 3 and kc % 2 == 1:
                        emit_up((kc - 3) // 2)
                    if kc == NT_K - 1 and not FP8UP:
                        emit_up(NP_K - 1, s_sel=(0,))

                def tail(h=h, ups=ups, es2=es2, emit_up=emit_up):
                    if FP8UP:
                        emit_up(NP_K - 1)
                        for j in range(NSP):
                            emit_uspan_epilogue(h, j, ups[j])
                    else:
                        for j in range(NSP):
                            nc.tensor.matmul(
                                ups[j],
                                qs2[NP_K - 1][:, 1, h * HB : h * HB + D + 1],
                                es2[NP_K - 1][:, 1, j * 512 : (j + 1) * 512],
                                start=False, stop=True,
                            )
                            emit_uspan_epilogue(h, j, ups[j])

                pend_tail = tail
                if h % 2 == 1:
                    pend_av = h // 2
            pend_tail()

            # tail: av(3) span j feeds out-proj stage B2 for its 4 row-tiles
            def emit_b2(st):
                op = psB.tile([128, E], f32, tag="misc", name="op")
                nc.tensor.matmul(
                    op, aoT[3][:, st * 128 : (st + 1) * 128], wot_sb[3],
                    start=True, stop=True,
                )
                ob = work.tile([128, E], f32, tag="ob")
                nc.vector.tensor_add(ob, op, partials[st])
                nc.sync.dma_start(out=out_dr[st * 128 : (st + 1) * 128, :], in_=ob)

            for j in range(NSP):
                emit_av_pair_span(pend_av, j)
                for st in range(4 * j, 4 * j + 4):
                    emit_b2(st)

    nc.compile()
    return nc


def _ensure_profile_hook():
    """Register the axon NTFF profile hook if the image's antenv lacks it."""
    import sys
    import types

    try:
        from antenv.axon_hooks import get_axon_ntff_profile_hook  # noqa: F401

        return True
    except ImportError:
        pass
    try:
        import antenv  # noqa: F401
        from trn_agent_boot.trn_boot import _ntff_profile_via_ctypes

        hook = _ntff_profile_via_ctypes("/opt/axon/libaxon_pjrt.so")
        if hook is None:
            return False
        mod = types.ModuleType("antenv.axon_hooks")
        mod._hook = hook
        mod.get_axon_ntff_profile_hook = lambda: mod._hook
        mod.set_axon_ntff_profile_hook = lambda h: setattr(mod, "_hook", h)
        sys.modules["antenv.axon_hooks"] = mod
        return True
    except Exception as e:  # pragma: no cover
        print(f"profile hook unavailable: {e}")
        return False


def _host_prep(queries, Wq, Wk, Wv, Wo, bo):
    q = np.asarray(queries, dtype=np.float32)
    Wq = np.asarray(Wq, dtype=np.float32)
    Wk = np.asarray(Wk, dtype=np.float32)
    Wv = np.asarray(Wv, dtype=np.float32)
    Wo = np.asarray(Wo, dtype=np.float32)
    bo = np.asarray(bo, dtype=np.float32)
    qdt = ml_dtypes.float8_e4m3 if FP8UP else BF16

    A = ((1.0 / np.sqrt(D)) * (Wq.T @ Wk)).astype(BF16)
    WvT = np.ascontiguousarray(Wv.T).astype(BF16)
    WoT = np.ascontiguousarray(Wo.T).astype(BF16)
    bob = np.ascontiguousarray(np.broadcast_to(bo, (128, E))).astype(np.float32)

    qb = q.reshape(B, S, H, D).astype(BF16)
    # padded per-head blocks with the ones column, in the attn@q dtype
    qp = np.zeros((B, S, H, HB), dtype=qdt)
    qp[..., :D] = qb.astype(qdt)
    qp[..., D] = 1.0
    qp = qp.reshape(B, S, H * HB)

    in_maps = []
    for c in range(8):
        b, half = divmod(c, 2)
        own = slice(half * SH, (half + 1) * SH)
        oth = slice((1 - half) * SH, (2 - half) * SH)
        # chunk-pair packing: row kp*128+p = [chunk 2kp row p | chunk 2kp+1 row p]
        qcat = np.concatenate([qp[b, own], qp[b, oth]], axis=0)  # [S, H*HB]
        qpin = np.ascontiguousarray(
            qcat.reshape(NP_K, 2, 128, H * HB)
            .transpose(0, 2, 1, 3)
            .reshape(SH, 2 * H * HB)
        )
        # transposed q, own-half columns first: [S, H, D] -> [E, S]
        qt = np.concatenate([qb[b, own], qb[b, oth]], axis=0)
        qt = np.ascontiguousarray(qt.transpose(1, 2, 0).reshape(E, S))
        in_maps.append(
            {
                "qpin": qpin,
                "qtin": qt,
                "a_bf": A,
                "wvt": WvT,
                "wot": WoT,
                "bob": bob,
            }
        )
    return in_maps


def kernel(queries, keys, values, Wq, Wk, Wv, Wo, bo):
    global LAST_EXEC_NS, LAST_RESULTS
    import concourse.bass_utils as bass_utils
    from concourse.bass_utils import run_bass_kernel_spmd

    in_maps = _host_prep(queries, Wq, Wk, Wv, Wo, bo)

    nc = _build_program()
    profile = bool(int(os.environ.get("KERNEL_PROFILE", "0")))
    if profile:
        profile = _ensure_profile_hook()
        # Keep profile artifacts local; no remote artifact store here.
        bass_utils.upload_artifacts = lambda tmpdir: tmpdir
    try:
        res = run_bass_kernel_spmd(nc, in_maps, list(range(8)), trace=profile)
    except Exception:
        if not profile:
            raise
        import traceback

        traceback.print_exc()
        print("profiled run failed; retrying without trace")
        res = run_bass_kernel_spmd(nc, in_maps, list(range(8)), trace=False)
    LAST_EXEC_NS = res.exec_time_ns
    LAST_RESULTS = res

    out = np.empty((B, S, E), dtype=np.float32)
    for c in range(8):
        b, half = divmod(c, 2)
        out[b, half * SH : (half + 1) * SH] = res.results[c]["out"]
    return out


# revision 31
# speedup vs baseline: 1.0080x; 1.0080x over previous
"""Trainium2 Bass kernel for the (faithfully buggy) multi-head attention module.

Reference math (k = v = q due to the reference's reshape bug):
    q  = queries.reshape(B, S, H, D)
    qp = q @ Wq.T ; kp = q @ Wk.T ; vp = q @ Wv.T        (per-head, shared W)
    sim = qp @ kp.T / sqrt(D) ; attn = softmax(sim)
    out = (attn @ vp).reshape(B, S, E) @ Wo.T + bo

Folded form computed here (algebraically identical):
    A   = (1/sqrt(D)) * Wq.T @ Wk          ->  sim = q @ A @ q.T
    u   = attn @ q ;  av = u @ Wv.T        ->  attn @ vp == (attn @ q) @ Wv.T
    out = concat_h(av) @ Wo.T + bo

Sharding: 8 cores = (4 batches) x (2 halves of the 2048 query rows).
Each core computes its 1024 output rows for all 8 heads; keys/values span
the full 2048 rows of the core's batch. No collectives.

On-chip dataflow stays in the "transposed domain" (head_dim on
partitions) so no attention-matrix transposes are ever needed:
    qT[d, k]      : host-prepared transposed q (qtin, bf16)
    tT  = A-lhsT @ qT(own cols)                      [d', q]
    scT = qT(k-chunk)-lhsT @ tT                      [k, q]   (PSUM)
    eS  = exp(scT)  on ACT                           [k, q]   (SBUF)
    uT  = [q_chunk | ones]-lhsT @ eS                 [d'+1, q] (PSUM accum
          over k-chunks; row 64 = softmax denominator via the ones col)
    ut  = uT[:64, :] * bcast(recip(uT[64, :]))       (normalize, DVE+GPSIMD)
    avT = WvT-lhsT @ ut                              [d, q]  (head pairs
          packed into one PSUM tile at row offsets 0/64)
    out = aoT-pair-chunks-lhsT @ WoT-chunks (+ bo)   [s, e]

Matmuls run in bf16 (4x the fp32 PE rate) with fp32 PSUM accumulation.
With FP8UP=1 (default), the attn@q contraction (uT) runs in fp8-e4m3
DoubleRow mode — k-chunk PAIRS are contracted per instruction at 2x the
bf16 row rate (4x overall) — exp outputs are written as fp8 directly and
the q chunks arrive host-packed as [128, 2, H*HB] pair tiles.

Scores and attn@q are software-pipelined at k-chunk granularity so the
ACT engine (~1.1us per 128x1024 exp) and the PE stay concurrently busy;
each pair's Wv projection + softmax-normalize chain is deferred into the
next head's dense scores phase so it never bubbles the PE.
"""

import os

import numpy as np
import ml_dtypes

B, S, E = 4, 2048, 512
H, D = 8, 64
SH = S // 2          # rows per core
HB = D + 2           # per-head block: 64 q cols, 1 ones col, 1 pad (alignment)
NT_Q = SH // 128     # 8 own-row tiles
NT_K = S // 128      # 16 k chunks
NP_K = NT_K // 2     # 8 k-chunk pairs
NSP = SH // 512      # 2 q spans of 512
BF16 = ml_dtypes.bfloat16

FP8UP = bool(int(os.environ.get("KERNEL_FP8UP", "0")))

LAST_EXEC_NS = None
LAST_RESULTS = None


def _build_program():
    import concourse.bass as bass  # noqa: F401
    import concourse.mybir as mybir
    import concourse.tile as tile
    from concourse import bacc

    f32 = mybir.dt.float32
    bf = mybir.dt.bfloat16
    qdt = mybir.dt.float8e4 if FP8UP else bf
    DR = mybir.MatmulPerfMode.DoubleRow

    nc = bacc.Bacc("TRN2", target_bir_lowering=False, debug=False)

    # q chunk-pair tiles: row kp*128+p holds [chunk 2kp row p | chunk 2kp+1 row p]
    qpin = nc.dram_tensor("qpin", [SH, 2 * H * HB], qdt, kind="ExternalInput").ap()
    qtin = nc.dram_tensor("qtin", [E, S], bf, kind="ExternalInput").ap()
    a_dr = nc.dram_tensor("a_bf", [D, D], bf, kind="ExternalInput").ap()
    wvt_dr = nc.dram_tensor("wvt", [D, D], bf, kind="ExternalInput").ap()
    wot_dr = nc.dram_tensor("wot", [E, E], bf, kind="ExternalInput").ap()
    bob_dr = nc.dram_tensor("bob", [128, E], f32, kind="ExternalInput").ap()
    out_dr = nc.dram_tensor("out", [SH, E], f32, kind="ExternalOutput").ap()

    with tile.TileContext(nc) as tc:
        with (
            tc.tile_pool(name="singles", bufs=1) as singles,
            tc.tile_pool(name="work", bufs=3) as work,
            tc.tile_pool(name="es", bufs=10) as espool,
            tc.tile_pool(name="psS", bufs=2, space="PSUM") as psS,
            tc.tile_pool(name="psU", bufs=2, space="PSUM") as psU,
            tc.tile_pool(name="psB", bufs=2, space="PSUM") as psB,
        ):
            # critical-path inputs first: A, first heads' qT, q chunk pairs
            a_sb = singles.tile([D, D], bf, tag="a_sb")
            nc.sync.dma_start(out=a_sb, in_=a_dr)
            qT = []
            for h in range(H):
                qT.append(singles.tile([D, S], bf, tag=f"qT{h}", name=f"qT{h}"))
            for h in range(2):
                for r in range(0, D, 16):
                    nc.sync.dma_start(
                        out=qT[h][r : r + 16, :],
                        in_=qtin[h * D + r : h * D + r + 16, :],
                    )
            qs2 = []
            for kp in range(NP_K):
                t = singles.tile([128, 2, H * HB], qdt, tag=f"qs{kp}", name=f"qs{kp}")
                if kp < 2:
                    for r in range(0, 128, 64):
                        nc.sync.dma_start(
                            out=t[r : r + 64, :, :],
                            in_=qpin[kp * 128 + r : kp * 128 + r + 64, :],
                        )
                else:
                    nc.sync.dma_start(out=t, in_=qpin[kp * 128 : (kp + 1) * 128, :])
                qs2.append(t)
            for h in range(2, H):
                nc.sync.dma_start(out=qT[h], in_=qtin[h * D : (h + 1) * D, :])

            # PE warm-up burst: dependency-free matmuls issued while input
            # DMAs stream, so the HAM clock gate opens before real work.
            wsc = singles.tile([128, 128], bf, tag="wsc")
            nc.vector.memset(wsc, 0.0)
            for i in range(16):
                wps = psB.tile([128, 128], f32, tag="misc", name="wps")
                nc.tensor.matmul(wps, wsc, wsc, start=True, stop=True)

            wvt_sb = singles.tile([D, D], bf, tag="wvt_sb")
            nc.sync.dma_start(out=wvt_sb, in_=wvt_dr)
            bob_sb = singles.tile([128, E], f32, tag="bob_sb")
            nc.sync.dma_start(out=bob_sb, in_=bob_dr)
            wot_sb = []
            for c in range(4):
                w = singles.tile([128, E], bf, tag=f"wot{c}", name=f"wot{c}")
                nc.sync.dma_start(out=w, in_=wot_dr[c * 128 : (c + 1) * 128, :])
                wot_sb.append(w)

            # attention outputs, head-PAIR packed: aoT[c][0:64] = head 2c,
            # aoT[c][64:128] = head 2c+1 (rows = e' = h*64+d).
            aoT = []
            for c in range(4):
                aoT.append(singles.tile([128, SH], bf, tag=f"aoT{c}", name=f"aoT{c}"))

            # per-head normalized uT spans, held until the pair's Wv matmul
            ut_tiles = {}

            def emit_uspan_epilogue(h, j, upj):
                # drain PSUM immediately, then normalize off the PE critical
                # path: ut = uT[:64] * bcast(1 / uT[64])
                uu = work.tile([65, 512], f32, tag="uu", bufs=4, name="uu")
                nc.vector.tensor_copy(uu, upj)
                d0 = work.tile([1, 512], f32, tag="d0", bufs=4, name="d0")
                nc.sync.dma_start(out=d0, in_=uu[64:65, :])
                rb = work.tile([D, 512], f32, tag="rb", bufs=4, name="rb")
                nc.gpsimd.partition_broadcast(rb, d0[0:1, :])
                rr = work.tile([D, 512], f32, tag="rr", bufs=4, name="rr")
                nc.vector.reciprocal_approx_fast(out=rr, in_=rb)
                ut = work.tile([D, 512], bf, tag="ut", bufs=6, name="ut")
                nc.vector.tensor_tensor(ut, uu[0:D, :], rr, mybir.AluOpType.mult)
                ut_tiles[(h, j)] = ut

            def emit_av_pair_span(hp, j):
                # Wv projection for both heads of the pair, packed in PSUM
                # rows 0:64 / 64:128, then one copy into the aoT pair tile.
                avp = psB.tile([128, 512], f32, tag="misc", name="avp")
                for hh in range(2):
                    nc.tensor.matmul(
                        avp[hh * D : (hh + 1) * D, :],
                        wvt_sb,
                        ut_tiles.pop((2 * hp + hh, j)),
                        start=True, stop=True,
                    )
                nc.vector.tensor_copy(aoT[hp][:, j * 512 : (j + 1) * 512], avp)

            def emit_tT_span(h, tts, j):
                # tT = (q_own @ A).T  [d', q]
                tp = psB.tile([D, 512], f32, tag="misc", name="tp")
                nc.tensor.matmul(
                    tp, a_sb, qT[h][:, j * 512 : (j + 1) * 512],
                    start=True, stop=True,
                )
                nc.vector.tensor_copy(tts[:, j * 512 : (j + 1) * 512], tp)

            # out-proj stage A: head pairs 0/1 contribution (+ bias), kept in
            # SBUF partials so only pairs 2/3 remain for the kernel tail.
            partials = {}

            def emit_outproj_b1_st(st):
                # accumulate head pair 2 onto the stage-A partial
                op = psB.tile([128, E], f32, tag="misc", name="opb")
                nc.tensor.matmul(
                    op, aoT[2][:, st * 128 : (st + 1) * 128], wot_sb[2],
                    start=True, stop=True,
                )
                nc.vector.tensor_add(partials[st], op, partials[st])

            def emit_outproj_a_st(st):
                op = psB.tile([128, E], f32, tag="misc", name="opa")
                for c in range(2):
                    nc.tensor.matmul(
                        op, aoT[c][:, st * 128 : (st + 1) * 128], wot_sb[c],
                        start=(c == 0), stop=(c == 1),
                    )
                pt = singles.tile([128, E], f32, tag=f"pt{st}", name=f"pt{st}")
                nc.vector.tensor_add(pt, op, bob_sb)
                partials[st] = pt

            # Software pipeline over heads: scores(h)/exp(h) interleaved with
            # attn@q of the same head lagging the pair's exp by one chunk;
            # each pair's Wv projection and out-proj stage A are deferred a
            # full head (ample slack for their DVE/DMA chains), and the next
            # head's tT is emitted mid-phase so head boundaries stay tight.
            pend_av = None
            pend_tail = None
            tts_all = [work.tile([D, SH], bf, tag=f"tts{h % 2}", name=f"tts{h}")
                       for h in range(H)]
            for j in range(NSP):
                emit_tT_span(0, tts_all[0], j)
            for h in range(H):
                tts = tts_all[h]
                es2 = []
                ups = [
                    psU.tile([D + 1, 512], f32, tag="up", name=f"up{j}")
                    for j in range(NSP)
                ]

                def emit_up(kp, s_sel=(0, 1), es2=es2, ups=ups, h=h):
                    if FP8UP:
                        for j in range(NSP):
                            nc.tensor.matmul(
                                ups[j],
                                qs2[kp][:, :, h * HB : h * HB + D + 1],
                                es2[kp][:, :, j * 512 : (j + 1) * 512],
                                start=(kp == 0), stop=(kp == NP_K - 1),
                                perf_mode=DR,
                            )
                    else:
                        for s in s_sel:
                            for j in range(NSP):
                                nc.tensor.matmul(
                                    ups[j],
                                    qs2[kp][:, s, h * HB : h * HB + D + 1],
                                    es2[kp][:, s, j * 512 : (j + 1) * 512],
                                    start=(kp == 0 and s == 0),
                                    stop=(kp == NP_K - 1 and s == 1),
                                )

                for kc in range(NT_K):
                    if kc == 1 and pend_tail is not None:
                        pend_tail()
                        pend_tail = None
                    if kc == 10 and pend_av is not None:
                        for j in range(NSP):
                            emit_av_pair_span(pend_av, j)
                        pend_av = None
                    if kc == 12 and h + 1 < H:
                        for j in range(NSP):
                            emit_tT_span(h + 1, tts_all[h + 1], j)
                    if h == 6 and kc in (8, 10, 12, 14):
                        st0 = (kc - 8)
                        emit_outproj_a_st(st0)
                        emit_outproj_a_st(st0 + 1)
                    if h == 7 and kc in (8, 10, 12, 14):
                        st0 = (kc - 8)
                        emit_outproj_b1_st(st0)
                        emit_outproj_b1_st(st0 + 1)
                    kp, s = divmod(kc, 2)
                    if s == 0:
                        es2.append(
                            espool.tile([128, 2, SH], qdt, tag="es", name=f"es{kp}")
                        )
                    sc = psS.tile([128, SH], f32, tag="sc")
                    for j in range(NSP):
                        nc.tensor.matmul(
                            sc[:, j * 512 : (j + 1) * 512],
                            qT[h][:, kc * 128 : (kc + 1) * 128],
                            tts[:, j * 512 : (j + 1) * 512],
                            start=True, stop=True,
                        )
                    nc.scalar.activation(
                        es2[kp][:, s, :], sc, mybir.ActivationFunctionType.Exp
                    )
                    # attn@q for an earlier pair, lagging its exp by one chunk
                    if kc >= 3 and kc % 2 == 1:
                        emit_up((kc - 3) // 2)
                    if kc == NT_K - 1 and not FP8UP:
                        emit_up(NP_K - 1, s_sel=(0,))

                def tail(h=h, ups=ups, emit_up=emit_up):
                    if FP8UP:
                        emit_up(NP_K - 1)
                    else:
                        emit_up(NP_K - 1, s_sel=(1,))
                    for j in range(NSP):
                        emit_uspan_epilogue(h, j, ups[j])

                pend_tail = tail
                if h % 2 == 1:
                    pend_av = h // 2
            pend_tail()

            # tail: av(3) span j feeds out-proj stage B2 for its 4 row-tiles
            def emit_b2(st):
                op = psB.tile([128, E], f32, tag="misc", name="op")
                nc.tensor.matmul(
                    op, aoT[3][:, st * 128 : (st + 1) * 128], wot_sb[3],
                    start=True, stop=True,
                )
                ob = work.tile([128, E], f32, tag="ob")
                nc.vector.tensor_add(ob, op, partials[st])
                nc.sync.dma_start(out=out_dr[st * 128 : (st + 1) * 128, :], in_=ob)

            for j in range(NSP):
                emit_av_pair_span(pend_av, j)
                for st in range(4 * j, 4 * j + 4):
                    emit_b2(st)

    nc.compile()
    return nc


def _ensure_profile_hook():
    """Register the axon NTFF profile hook if the image's antenv lacks it."""
    import sys
    import types

    try:
        from antenv.axon_hooks import get_axon_ntff_profile_hook  # noqa: F401

        return True
    except ImportError:
        pass
    try:
        import antenv  # noqa: F401
        from trn_agent_boot.trn_boot import _ntff_profile_via_ctypes

        hook = _ntff_profile_via_ctypes("/opt/axon/libaxon_pjrt.so")
        if hook is None:
            return False
        mod = types.ModuleType("antenv.axon_hooks")
        mod._hook = hook
        mod.get_axon_ntff_profile_hook = lambda: mod._hook
        mod.set_axon_ntff_profile_hook = lambda h: setattr(mod, "_hook", h)
        sys.modules["antenv.axon_hooks"] = mod
        return True
    except Exception as e:  # pragma: no cover
        print(f"profile hook unavailable: {e}")
        return False


def _host_prep(queries, Wq, Wk, Wv, Wo, bo):
    q = np.asarray(queries, dtype=np.float32)
    Wq = np.asarray(Wq, dtype=np.float32)
    Wk = np.asarray(Wk, dtype=np.float32)
    Wv = np.asarray(Wv, dtype=np.float32)
    Wo = np.asarray(Wo, dtype=np.float32)
    bo = np.asarray(bo, dtype=np.float32)
    qdt = ml_dtypes.float8_e4m3 if FP8UP else BF16

    A = ((1.0 / np.sqrt(D)) * (Wq.T @ Wk)).astype(BF16)
    WvT = np.ascontiguousarray(Wv.T).astype(BF16)
    WoT = np.ascontiguousarray(Wo.T).astype(BF16)
    bob = np.ascontiguousarray(np.broadcast_to(bo, (128, E))).astype(np.float32)

    qb = q.reshape(B, S, H, D).astype(BF16)
    # padded per-head blocks with the ones column, in the attn@q dtype
    qp = np.zeros((B, S, H, HB), dtype=qdt)
    qp[..., :D] = qb.astype(qdt)
    qp[..., D] = 1.0
    qp = qp.reshape(B, S, H * HB)

    in_maps = []
    for c in range(8):
        b, half = divmod(c, 2)
        own = slice(half * SH, (half + 1) * SH)
        oth = slice((1 - half) * SH, (2 - half) * SH)
        # chunk-pair packing: row kp*128+p = [chunk 2kp row p | chunk 2kp+1 row p]
        qcat = np.concatenate([qp[b, own], qp[b, oth]], axis=0)  # [S, H*HB]
        qpin = np.ascontiguousarray(
            qcat.reshape(NP_K, 2, 128, H * HB)
            .transpose(0, 2, 1, 3)
            .reshape(SH, 2 * H * HB)
        )
        # transposed q, own-half columns first: [S, H, D] -> [E, S]
        qt = np.concatenate([qb[b, own], qb[b, oth]], axis=0)
        qt = np.ascontiguousarray(qt.transpose(1, 2, 0).reshape(E, S))
        in_maps.append(
            {
                "qpin": qpin,
                "qtin": qt,
                "a_bf": A,
                "wvt": WvT,
                "wot": WoT,
                "bob": bob,
            }
        )
    return in_maps


def kernel(queries, keys, values, Wq, Wk, Wv, Wo, bo):
    global LAST_EXEC_NS, LAST_RESULTS
    import concourse.bass_utils as bass_utils
    from concourse.bass_utils import run_bass_kernel_spmd

    in_maps = _host_prep(queries, Wq, Wk, Wv, Wo, bo)

    nc = _build_program()
    profile = bool(int(os.environ.get("KERNEL_PROFILE", "0")))
    if profile:
        profile = _ensure_profile_hook()
        # Keep profile artifacts local; no remote artifact store here.
        bass_utils.upload_artifacts = lambda tmpdir: tmpdir
    try:
        res = run_bass_kernel_spmd(nc, in_maps, list(range(8)), trace=profile)
    except Exception:
        if not profile:
            raise
        import traceback

        traceback.print_exc()
        print("profiled run failed; retrying without trace")
        res = run_bass_kernel_spmd(nc, in_maps, list(range(8)), trace=False)
    LAST_EXEC_NS = res.exec_time_ns
    LAST_RESULTS = res

    out = np.empty((B, S, E), dtype=np.float32)
    for c in range(8):
        b, half = divmod(c, 2)
        out[b, half * SH : (half + 1) * SH] = res.results[c]["out"]
    return out
